# revision 18
# baseline (speedup 1.0000x reference)
"""Trainium2 Bass kernel for nn_LinearSoftmaxAttention (second-order linear attention).

Math (per batch n, head h; L == S, D == M):
    Q = LN(queries)                       [L,D]
    K = LN(keys) / (3*sqrt(D)) * klen     [S,D]
    psumA  = Kn^T [V | 1 | 1 | Kn]        [D, 2D+2] = [KV | Ksum | Ksum | KK]
    Qsum2  = Qn^T Qn                      [D,D]
    psumC  = Q @ sbA                      [L, 2D+2] = [order1 | norm1 | . | u]
    psumD  = K @ (0.5*Qsum2)              [S,D]
    ch     = rowsum(psumD * K)            [S]     (= 0.5 * c)
    nrm    = norm1 + 0.5*rowsum(u * Q)    [L]
    out    = (order1 + ch*V) / nrm[:,None]

Sharding: one (n,h) pair per NeuronCore -> 8 heads over 8 cores, no collectives.
All matmuls in f16 (inputs are converted host-side; rel-err budget is 2e-2).
Transposes of [Kn|Qn] are done per 128-row chunk on the PE ([128,64] -> [64,128]),
giving kT on partitions 0:32 and qT on partitions 32:64 so that the d-contraction
matmuls (C and D) run as row-group-tiled matmuls without any cross-partition moves.
"""

from contextlib import ExitStack

import numpy as np
import ml_dtypes

import concourse.bacc as bacc
import concourse.mybir as mybir
from concourse import tile
from concourse.bass_utils import run_bass_kernel_spmd
from concourse.masks import make_identity

# Problem constants (hardcoded per harness contract).
L = 512  # query length == key length
D = 32   # head dim == value dim
H = 8    # heads
P = 128  # SBUF partitions
T = L // P  # 4 row-chunks of 128
ALPHA = 3.0
LN_EPS = 1e-5
_INV_C2 = float(ALPHA * ALPHA * D)  # 1/c^2 = 288  (K scale folded into sqrt arg)

# dram input layout (f16 cols): k | q | klen | v
_KOFF, _QOFF, _LOFF, _VOFF = 0, T * D, 2 * T * D, 2 * T * D + T
NCOL = 3 * T * D + T  # 388

# work tile free-dim layout per chunk (f16, 4B-aligned slices):
#   [ V(0:32) | one(32) one(33) | Kn(34:66) | Qn(66:98) | pad ]
_WV, _W1, _WK, _WQ, _WW = 0, D, D + 2, 2 * D + 2, 3 * D + 4  # 0,32,34,66,100
CAT = 2 * D + 2  # 66: A-matmul rhs/psum width [KV | Ksum Ksum | KK]

_SUB = mybir.AluOpType.subtract
_MUL = mybir.AluOpType.mult
_ADD = mybir.AluOpType.add
_BYP = mybir.AluOpType.bypass
_AX = mybir.AxisListType.X


def _emit(ctx: ExitStack, tc: tile.TileContext, in_d, inv_d, out_d):
    nc = tc.nc
    f32 = mybir.dt.float32
    f16 = mybir.dt.float16
    ACT = mybir.ActivationFunctionType

    consts = ctx.enter_context(tc.tile_pool(name="consts", bufs=1))
    sbuf = ctx.enter_context(tc.tile_pool(name="sbuf", bufs=1))
    psum = ctx.enter_context(tc.tile_pool(name="psum", bufs=1, space="PSUM"))
    psum_tr = ctx.enter_context(tc.tile_pool(name="psum_tr", bufs=2, space="PSUM"))

    identity = consts.tile([P, P], f16)
    make_identity(nc, identity[:])
    dummy = consts.tile([P, 1], f32)
    eps_b = consts.tile([P, 1], f32)
    # Dependency-free Sqrt issued first so Bacc's hoisted act-table load
    # overlaps the input DMA instead of sitting on the LN critical path.
    nc.scalar.activation(dummy[:], nc.const_aps.tensor(0.0, (P, 1)), ACT.Sqrt)
    nc.gpsimd.memset(eps_b[:], LN_EPS)

    # ---- input DMAs; raw = [k | q | klen] fp32, v (f16) lands in work ----
    raw = sbuf.tile([P, 2 * T * D + T], f32)  # [128, 260]
    work = sbuf.tile([P, T, _WW], f16)
    nc.gpsimd.memset(work[:, :, _W1 : _W1 + 2], 1.0)
    nc.sync.dma_start(raw[:], in_d[:])
    nc.scalar.dma_start(
        work[:, :, _WV : _WV + D],
        inv_d.rearrange("p (t d) -> p t d", d=D),
    )
    k_raw = raw[:, 0 : T * D].rearrange("p (t d) -> p t d", d=D)
    q_raw = raw[:, T * D : 2 * T * D].rearrange("p (t d) -> p t d", d=D)
    klen = raw[:, 2 * T * D : 2 * T * D + T]  # [128, 4] f32
    kq_g = raw[:, 0 : 2 * T * D].rearrange("p (g d) -> p g d", d=D)  # 8 groups

    # ---- LN stats: s = rowsum(x), ss = rowsum(x^2) per 32-col group ----
    # (k groups 0:4, q groups 4:8). var*32 = ss - s^2/32.
    G = 2 * T
    sq = sbuf.tile([P, 2 * T * D], f16)
    nc.scalar.activation(sq[:], raw[:, 0 : 2 * T * D], ACT.Square)
    s_ = sbuf.tile([P, G], f32)
    ss = sbuf.tile([P, G], f32)
    v32 = sbuf.tile([P, G], f32)
    rs = sbuf.tile([P, G], f32)   # 1/std (k part includes klen/alpha factors)
    nmu = sbuf.tile([P, G], f32)  # -mean * rs
    std = sbuf.tile([P, G], f32)
    klenp = sbuf.tile([P, T], f32)  # klen / (alpha*sqrt(D))
    nc.vector.tensor_reduce(s_[:], kq_g, axis=_AX, op=_ADD)
    nc.gpsimd.tensor_scalar(out=klenp[:], in0=klen, scalar1=1.0 / _INV_C2**0.5,
                            scalar2=None, op0=_MUL)
    # v32 scratch = -(s/32)*s
    nc.vector.scalar_tensor_tensor(out=v32[:], in0=s_[:], scalar=-1.0 / D,
                                   in1=s_[:], op0=_MUL, op1=_MUL)
    nc.vector.tensor_reduce(ss[:], sq[:].rearrange("p (g d) -> p g d", d=D),
                            axis=_AX, op=_ADD)
    nc.vector.tensor_add(v32[:], v32[:], ss[:])
    # std = sqrt(var + eps) for all 8 groups in one ACT call
    nc.scalar.activation(std[:], v32[:], ACT.Sqrt, scale=1.0 / D, bias=eps_b[:])
    nc.vector.reciprocal(rs[:], std[:])
    nc.vector.tensor_mul(rs[:, 0:T], rs[:, 0:T], klenp[:])
    nc.vector.scalar_tensor_tensor(out=nmu[:], in0=s_[:], scalar=-1.0 / D,
                                   in1=rs[:], op0=_MUL, op1=_MUL)

    # ---- LN applies x*rs + nmu: split ACT[k0,k1] DVE[k2,q2,q3] GpS[q0,q1] ----
    def apply_k(eng, t):
        if eng is nc.scalar:
            nc.scalar.activation(work[:, t, _WK : _WK + D], k_raw[:, t, :],
                                 ACT.Identity, scale=rs[:, t : t + 1],
                                 bias=nmu[:, t : t + 1])
        else:
            eng.tensor_scalar(out=work[:, t, _WK : _WK + D], in0=k_raw[:, t, :],
                              scalar1=rs[:, t : t + 1],
                              scalar2=nmu[:, t : t + 1], op0=_MUL, op1=_ADD)

    def apply_q(eng, t):
        eng.tensor_scalar(out=work[:, t, _WQ : _WQ + D], in0=q_raw[:, t, :],
                          scalar1=rs[:, T + t : T + t + 1],
                          scalar2=nmu[:, T + t : T + t + 1],
                          op0=_MUL, op1=_ADD)

    apply_k(nc.scalar, 0)
    apply_k(nc.scalar, 1)
    apply_k(nc.vector, 2)
    apply_k(nc.vector, 3)
    apply_q(nc.gpsimd, 0)
    apply_q(nc.gpsimd, 1)
    apply_q(nc.vector, 2)
    apply_q(nc.gpsimd, 3)

    # ---- PE: A (into psum partitions 32:64), B, per-chunk [Kn|Qn] transpose ----
    psumA = psum.tile([2 * D, CAT], f32)      # rows 32:64 used
    psumB = psum.tile([D, D], f32)
    kqT = sbuf.tile([2 * D, T, P], f16)      # rows 0:32 kT, 32:64 qT
    pT = [None] * T
    for t in range(T):
        st, sp = (t == 0), (t == T - 1)
        nc.tensor.matmul(psumA[D : 2 * D, :], work[:, t, _WK : _WK + D],
                         work[:, t, 0:CAT], start=st, stop=sp,
                         tile_position=(0, D))
        nc.tensor.matmul(psumB[:], work[:, t, _WQ : _WQ + D],
                         work[:, t, _WQ : _WQ + D], start=st, stop=sp)
        ptile = psum_tr.tile([2 * D, P], f16, tag="ptr")
        pT[t] = ptile
        nc.tensor.transpose(ptile[:], work[:, t, _WK : _WK + 2 * D], identity[:])

    # evacuations: sbA (f16, partitions 32:64) on DVE; sbB = 0.5*Qsum2 on ACT;
    # kqT chunk copies split ACT/DVE
    sbA = sbuf.tile([2 * D, CAT], f16)
    sbB = sbuf.tile([D, D], f16)
    nc.scalar.activation(sbB[:], psumB[:], ACT.Copy, scale=0.5)
    nc.scalar.copy(kqT[:, 0, :], pT[0][:])
    nc.vector.tensor_copy(kqT[:, 1, :], pT[1][:])
    nc.vector.tensor_copy(sbA[D : 2 * D, :], psumA[D : 2 * D, :])
    nc.scalar.copy(kqT[:, 2, :], pT[2][:])
    nc.vector.tensor_copy(kqT[:, 3, :], pT[3][:])

    # ---- PE: D then C (row-group-tiled over d) ----
    psumD = psum.tile([P, T, D], f32)
    psumC = psum.tile([P, T, CAT], f32)
    for t in range(T):
        nc.tensor.matmul(psumD[:, t, :], kqT[0:D, t, :], sbB[:],
                         start=True, stop=True)
    for t in range(T):
        nc.tensor.matmul(psumC[:, t, :], kqT[D : 2 * D, t, :], sbA[D : 2 * D, :],
                         start=True, stop=True, tile_position=(D, 0))

    # ---- epilogue ----
    # ch = rowsum(psumD * Kn)  (starts while C matmuls still run)
    e1 = sbuf.tile([P, T, D], f32)
    ch = sbuf.tile([P, T], f32)
    nc.vector.tensor_mul(e1[:], psumD[:], work[:, :, _WK : _WK + D])
    nc.vector.tensor_reduce(ch[:], e1[:], axis=_AX, op=_ADD)
    # nrm = norm1 + 0.5*rowsum(u * Qn); rnorm = 1/nrm
    e2 = sbuf.tile([P, T, D], f32)
    nrm = sbuf.tile([P, T], f32)
    nc.vector.tensor_mul(e2[:], psumC[:, :, _WK:CAT], work[:, :, _WQ : _WQ + D])
    nc.vector.tensor_reduce(nrm[:], e2[:], axis=_AX, op=_ADD)
    nc.vector.scalar_tensor_tensor(out=nrm[:], in0=nrm[:], scalar=0.5,
                                   in1=psumC[:, :, D], op0=_MUL, op1=_ADD)
    nc.vector.reciprocal(nrm[:], nrm[:])
    # out_t = (V*ch + order1) * rnorm ; DVE does the STT, ACT/GpSimd the
    # final scales, output DMA'd in two chunk-pairs.
    out_sb = sbuf.tile([P, T, D], f32)
    for t in range(T):
        s3 = sbuf.tile([P, D], f32, tag="epi_s3", bufs=2)
        nc.vector.scalar_tensor_tensor(
            out=s3[:], in0=work[:, t, _WV : _WV + D], scalar=ch[:, t : t + 1],
            in1=psumC[:, t, 0:D], op0=_MUL, op1=_ADD)
        if t % 2 == 0:
            nc.scalar.activation(out_sb[:, t, :], s3[:], ACT.Identity,
                                 scale=nrm[:, t : t + 1])
        else:
            nc.gpsimd.tensor_scalar(out=out_sb[:, t, :], in0=s3[:],
                                    scalar1=nrm[:, t : t + 1], scalar2=None,
                                    op0=_MUL)
        if t == 1:
            nc.sync.dma_start(out_d[:, 0 : 2 * D],
                              out_sb[:, 0:2, :].rearrange("p t d -> p (t d)"))
        elif t == 3:
            nc.sync.dma_start(out_d[:, 2 * D : 4 * D],
                              out_sb[:, 2:4, :].rearrange("p t d -> p (t d)"))


_CACHED = {}


def _build():
    if "nc" in _CACHED:
        return _CACHED["nc"]
    # Route every ACT func we use (Sqrt/Copy/Identity/Square) to the single
    # act-func-set that contains them all, so Bacc inserts ONE table load
    # instead of one per first-match set.
    import concourse.hw_specs as hw_specs
    orig_tables = hw_specs.get_activation_tables

    def _tables_one_set(module_arch):
        tabs = orig_tables(module_arch)
        keep = None
        for name, funcs in tabs.items():
            names = {str(f) for f in funcs}
            if any("Sqrt" in s and "Rsqrt" not in s for s in names):
                keep = name
                break
        if keep is None:
            return tabs
        shared = {
            mybir.ActivationFunctionType.Copy,
            mybir.ActivationFunctionType.Identity,
            mybir.ActivationFunctionType.Square,
        }
        return {
            name: (funcs if name == keep else funcs - shared)
            for name, funcs in tabs.items()
        }

    bacc.get_activation_tables = _tables_one_set
    try:
        nc = bacc.Bacc("TRN2", target_bir_lowering=False, debug=False,
                       num_devices=H)
        f32 = mybir.dt.float32
        f16 = mybir.dt.float16
        in_d = nc.dram_tensor("inp", [P, 2 * T * D + T], f32,
                              kind="ExternalInput")
        inv_d = nc.dram_tensor("inpv", [P, T * D], f16, kind="ExternalInput")
        out_d = nc.dram_tensor("out", [P, T * D], f32, kind="ExternalOutput")
        with tile.TileContext(nc) as tc:
            with ExitStack() as ctx:
                _emit(ctx, tc, in_d[:], inv_d[:], out_d[:])
        nc.compile()
    finally:
        bacc.get_activation_tables = orig_tables
    _CACHED["nc"] = nc
    return nc


def _pack(q, k, v, klen, h):
    # [512, 32] -> [128, 4*32] with col t*32+d = row t*128+p
    def rows(x):
        return np.ascontiguousarray(
            x.reshape(T, P, D).transpose(1, 0, 2).reshape(P, T * D))
    kl = np.ascontiguousarray(klen.reshape(T, P).T)  # [128, 4]
    kq = np.concatenate(
        [rows(k[0, :, h, :]), rows(q[0, :, h, :]), kl], axis=1)
    return (kq.astype(np.float32), rows(v[0, :, h, :]).astype(np.float16))


def kernel(queries, keys, values, attn_mask, query_lengths, key_lengths,
           _want_profile=False, **_ignored):
    nc = _build()
    q = np.asarray(queries, dtype=np.float32)
    k = np.asarray(keys, dtype=np.float32)
    v = np.asarray(values, dtype=np.float32)
    klen = np.asarray(key_lengths, dtype=np.float32)

    packed = [_pack(q, k, v, klen, h) for h in range(H)]
    in_maps = [{"inp": kq, "inpv": vv} for kq, vv in packed]
    res = run_bass_kernel_spmd(nc, in_maps, list(range(H)),
                               trace=_want_profile)
    # [128, 128] -> [512, 32]
    outs = [
        np.asarray(res.results[h]["out"], dtype=np.float32)
        .reshape(P, T, D).transpose(1, 0, 2).reshape(L, D)
        for h in range(H)
    ]
    out = np.stack(outs, axis=1)[None]
    if _want_profile:
        return out.astype(np.float32), res
    return out.astype(np.float32)


# revision 21
# speedup vs baseline: 1.0466x; 1.0466x over previous
"""Trainium2 Bass kernel for nn_LinearSoftmaxAttention (second-order linear attention).

Math (per batch n, head h; L == S, D == M):
    Q = LN(queries)                       [L,D]
    K = LN(keys) / (3*sqrt(D)) * klen     [S,D]
    psumA  = Kn^T [V | 1 | 1 | Kn]        [D, 2D+2] = [KV | Ksum | Ksum | KK]
    Qsum2  = Qn^T Qn                      [D,D]
    psumC  = Q @ sbA                      [L, 2D+2] = [order1 | norm1 | . | u]
    psumD  = K @ (0.5*Qsum2)              [S,D]
    ch     = rowsum(psumD * K)            [S]     (= 0.5 * c)
    nrm    = norm1 + 0.5*rowsum(u * Q)    [L]
    out    = (order1 + ch*V) / nrm[:,None]

Sharding: one (n,h) pair per NeuronCore -> 8 heads over 8 cores, no collectives.
All matmuls in f16 (inputs are converted host-side; rel-err budget is 2e-2).
Transposes of [Kn|Qn] are done per 128-row chunk on the PE ([128,64] -> [64,128]),
giving kT on partitions 0:32 and qT on partitions 32:64 so that the d-contraction
matmuls (C and D) run as row-group-tiled matmuls without any cross-partition moves.
"""

from contextlib import ExitStack

import numpy as np
import ml_dtypes

import concourse.bacc as bacc
import concourse.mybir as mybir
from concourse import tile
from concourse.bass_utils import run_bass_kernel_spmd
from concourse.masks import make_identity

# Problem constants (hardcoded per harness contract).
L = 512  # query length == key length
D = 32   # head dim == value dim
H = 8    # heads
P = 128  # SBUF partitions
T = L // P  # 4 row-chunks of 128
ALPHA = 3.0
LN_EPS = 1e-5
_INV_C2 = float(ALPHA * ALPHA * D)  # 1/c^2 = 288  (K scale folded into sqrt arg)

# dram input layout (f16 cols): k | q | klen | v
_KOFF, _QOFF, _LOFF, _VOFF = 0, T * D, 2 * T * D, 2 * T * D + T
NCOL = 3 * T * D + T  # 388

# work tile free-dim layout per chunk (f16, 4B-aligned slices):
#   [ V(0:32) | one(32) one(33) | Kn(34:66) | Qn(66:98) | pad ]
_WV, _W1, _WK, _WQ, _WW = 0, D, D + 2, 2 * D + 2, 3 * D + 4  # 0,32,34,66,100
CAT = 2 * D + 2  # 66: A-matmul rhs/psum width [KV | Ksum Ksum | KK]

_SUB = mybir.AluOpType.subtract
_MUL = mybir.AluOpType.mult
_ADD = mybir.AluOpType.add
_BYP = mybir.AluOpType.bypass
_AX = mybir.AxisListType.X


def _emit(ctx: ExitStack, tc: tile.TileContext, in_d, inv_d, out_d):
    nc = tc.nc
    f32 = mybir.dt.float32
    f16 = mybir.dt.float16
    ACT = mybir.ActivationFunctionType

    consts = ctx.enter_context(tc.tile_pool(name="consts", bufs=1))
    sbuf = ctx.enter_context(tc.tile_pool(name="sbuf", bufs=1))
    psum = ctx.enter_context(tc.tile_pool(name="psum", bufs=1, space="PSUM"))
    psum_tr = ctx.enter_context(tc.tile_pool(name="psum_tr", bufs=2, space="PSUM"))

    identity = consts.tile([P, P], f16)
    make_identity(nc, identity[:])
    dummy = consts.tile([P, 1], f32)
    eps_b = consts.tile([P, 1], f32)
    # Dependency-free Sqrt issued first so Bacc's hoisted act-table load
    # overlaps the input DMA instead of sitting on the LN critical path.
    nc.scalar.activation(dummy[:], nc.const_aps.tensor(0.0, (P, 1)), ACT.Sqrt)
    nc.gpsimd.memset(eps_b[:], LN_EPS)
    # HAM warm-up: dependency-free matmuls fill the DMA-wait window so the
    # PE clock-gate opens (1.2 -> 2.4 GHz) before the real matmuls run.
    warm_ps = psum.tile([D, P], f32)
    for _ in range(44):
        nc.tensor.matmul(warm_ps[:], identity[:, 0:D], identity[:],
                         start=True, stop=True)

    # ---- input DMAs; raw = [k | q | klen] fp32, v (f16) lands in work ----
    raw = sbuf.tile([P, 2 * T * D + T], f32)  # [128, 260]
    work = sbuf.tile([P, T, _WW], f16)
    nc.gpsimd.memset(work[:, :, _W1 : _W1 + 2], 1.0)
    nc.sync.dma_start(raw[:], in_d[:])
    nc.scalar.dma_start(
        work[:, :, _WV : _WV + D],
        inv_d.rearrange("p (t d) -> p t d", d=D),
    )
    k_raw = raw[:, 0 : T * D].rearrange("p (t d) -> p t d", d=D)
    q_raw = raw[:, T * D : 2 * T * D].rearrange("p (t d) -> p t d", d=D)
    klen = raw[:, 2 * T * D : 2 * T * D + T]  # [128, 4] f32
    kq_g = raw[:, 0 : 2 * T * D].rearrange("p (g d) -> p g d", d=D)  # 8 groups

    # ---- LN stats: s = rowsum(x), ss = rowsum(x^2) per 32-col group ----
    # (k groups 0:4, q groups 4:8). var*32 = ss - s^2/32.
    G = 2 * T
    sq = sbuf.tile([P, 2 * T * D], f16)
    nc.scalar.activation(sq[:], raw[:, 0 : 2 * T * D], ACT.Square)
    s_ = sbuf.tile([P, G], f32)
    ss = sbuf.tile([P, G], f32)
    v32 = sbuf.tile([P, G], f32)
    rs = sbuf.tile([P, G], f32)   # 1/std (k part includes klen/alpha factors)
    nmu = sbuf.tile([P, G], f32)  # -mean * rs
    std = sbuf.tile([P, G], f32)
    klenp = sbuf.tile([P, T], f32)  # klen / (alpha*sqrt(D))
    nc.vector.tensor_reduce(s_[:], kq_g, axis=_AX, op=_ADD)
    nc.gpsimd.tensor_scalar(out=klenp[:], in0=klen, scalar1=1.0 / _INV_C2**0.5,
                            scalar2=None, op0=_MUL)
    # v32 scratch = -(s/32)*s
    nc.vector.scalar_tensor_tensor(out=v32[:], in0=s_[:], scalar=-1.0 / D,
                                   in1=s_[:], op0=_MUL, op1=_MUL)
    nc.vector.tensor_reduce(ss[:], sq[:].rearrange("p (g d) -> p g d", d=D),
                            axis=_AX, op=_ADD)
    nc.vector.tensor_add(v32[:], v32[:], ss[:])
    # std = sqrt(var + eps) for all 8 groups in one ACT call
    nc.scalar.activation(std[:], v32[:], ACT.Sqrt, scale=1.0 / D, bias=eps_b[:])
    nc.vector.reciprocal(rs[:], std[:])
    nc.vector.tensor_mul(rs[:, 0:T], rs[:, 0:T], klenp[:])
    nc.vector.scalar_tensor_tensor(out=nmu[:], in0=s_[:], scalar=-1.0 / D,
                                   in1=rs[:], op0=_MUL, op1=_MUL)

    # ---- LN applies x*rs + nmu via free-dim-broadcast TTs (all chunks at
    # once): k half on DVE, q half on GpSimd; ACT helps with the adds ----
    tmpk = sbuf.tile([P, T, D], f32)
    tmpq = sbuf.tile([P, T, D], f32)
    rs_kb = rs[:, 0:T, None].broadcast_to((P, T, D))
    rs_qb = rs[:, T : 2 * T, None].broadcast_to((P, T, D))
    nmu_kb = nmu[:, 0:T, None].broadcast_to((P, T, D))
    nmu_qb = nmu[:, T : 2 * T, None].broadcast_to((P, T, D))
    nc.vector.tensor_mul(tmpk[:], k_raw, rs_kb)
    nc.gpsimd.tensor_mul(tmpq[:], q_raw, rs_qb)
    nc.vector.tensor_add(work[:, :, _WK : _WK + D], tmpk[:], nmu_kb)
    nc.gpsimd.tensor_add(work[:, :, _WQ : _WQ + D], tmpq[:], nmu_qb)

    # ---- PE: A (into psum partitions 32:64), B, per-chunk [Kn|Qn] transpose ----
    psumA = psum.tile([2 * D, CAT], f32)      # rows 32:64 used
    psumB = psum.tile([D, D], f32)
    kqT = sbuf.tile([2 * D, T, P], f16)      # rows 0:32 kT, 32:64 qT
    pT = [None] * T
    for t in range(T):
        st, sp = (t == 0), (t == T - 1)
        nc.tensor.matmul(psumA[D : 2 * D, :], work[:, t, _WK : _WK + D],
                         work[:, t, 0:CAT], start=st, stop=sp,
                         tile_position=(0, D))
        nc.tensor.matmul(psumB[:], work[:, t, _WQ : _WQ + D],
                         work[:, t, _WQ : _WQ + D], start=st, stop=sp)
        ptile = psum_tr.tile([2 * D, P], f16, tag="ptr")
        pT[t] = ptile
        nc.tensor.transpose(ptile[:], work[:, t, _WK : _WK + 2 * D], identity[:])

    # evacuations: sbA (f16, partitions 32:64) on DVE; sbB = 0.5*Qsum2 on ACT;
    # kqT chunk copies split ACT/DVE
    sbA = sbuf.tile([2 * D, CAT], f16)
    sbB = sbuf.tile([D, D], f16)
    nc.scalar.activation(sbB[:], psumB[:], ACT.Copy, scale=0.5)
    nc.scalar.copy(kqT[:, 0, :], pT[0][:])
    nc.vector.tensor_copy(kqT[:, 1, :], pT[1][:])
    nc.vector.tensor_copy(sbA[D : 2 * D, :], psumA[D : 2 * D, :])
    nc.scalar.copy(kqT[:, 2, :], pT[2][:])
    nc.vector.tensor_copy(kqT[:, 3, :], pT[3][:])

    # ---- PE: D then C (row-group-tiled over d) ----
    psumD = psum.tile([P, T, D], f32)
    psumC = psum.tile([P, T, CAT], f32)
    for t in range(T):
        nc.tensor.matmul(psumD[:, t, :], kqT[0:D, t, :], sbB[:],
                         start=True, stop=True)
    for t in range(T):
        nc.tensor.matmul(psumC[:, t, :], kqT[D : 2 * D, t, :], sbA[D : 2 * D, :],
                         start=True, stop=True, tile_position=(D, 0))

    # ---- epilogue ----
    # ch = rowsum(psumD * Kn)  (starts while C matmuls still run)
    e1 = sbuf.tile([P, T, D], f32)
    ch = sbuf.tile([P, T], f32)
    nc.vector.tensor_mul(e1[:], psumD[:], work[:, :, _WK : _WK + D])
    nc.vector.tensor_reduce(ch[:], e1[:], axis=_AX, op=_ADD)
    # nrm = norm1 + 0.5*rowsum(u * Qn); rnorm = 1/nrm
    e2 = sbuf.tile([P, T, D], f32)
    nrm = sbuf.tile([P, T], f32)
    nc.vector.tensor_mul(e2[:], psumC[:, :, _WK:CAT], work[:, :, _WQ : _WQ + D])
    nc.vector.tensor_reduce(nrm[:], e2[:], axis=_AX, op=_ADD)
    nc.vector.scalar_tensor_tensor(out=nrm[:], in0=nrm[:], scalar=0.5,
                                   in1=psumC[:, :, D], op0=_MUL, op1=_ADD)
    nc.vector.reciprocal(nrm[:], nrm[:])
    # out = (V*ch + order1) * rnorm, all chunks at once via broadcast TTs:
    # f1 = V*ch_b (GpSimd, as soon as ch lands), f2 = f1 + order1 (DVE),
    # out = f2 * rnorm_b (DVE), single output DMA.
    f1 = sbuf.tile([P, T, D], f32)
    f2 = sbuf.tile([P, T, D], f32)
    out_sb = sbuf.tile([P, T, D], f32)
    ch_b = ch[:, :, None].broadcast_to((P, T, D))
    nrm_b = nrm[:, :, None].broadcast_to((P, T, D))
    nc.gpsimd.tensor_mul(f1[:], work[:, :, _WV : _WV + D], ch_b)
    nc.vector.tensor_add(f2[:], f1[:], psumC[:, :, 0:D])
    nc.vector.tensor_mul(out_sb[:], f2[:], nrm_b)
    nc.sync.dma_start(out_d[:], out_sb[:].rearrange("p t d -> p (t d)"))


_CACHED = {}


def _build():
    if "nc" in _CACHED:
        return _CACHED["nc"]
    # Route every ACT func we use (Sqrt/Copy/Identity/Square) to the single
    # act-func-set that contains them all, so Bacc inserts ONE table load
    # instead of one per first-match set.
    import concourse.hw_specs as hw_specs
    orig_tables = hw_specs.get_activation_tables

    def _tables_one_set(module_arch):
        tabs = orig_tables(module_arch)
        keep = None
        for name, funcs in tabs.items():
            names = {str(f) for f in funcs}
            if any("Sqrt" in s and "Rsqrt" not in s for s in names):
                keep = name
                break
        if keep is None:
            return tabs
        shared = {
            mybir.ActivationFunctionType.Copy,
            mybir.ActivationFunctionType.Identity,
            mybir.ActivationFunctionType.Square,
        }
        return {
            name: (funcs if name == keep else funcs - shared)
            for name, funcs in tabs.items()
        }

    bacc.get_activation_tables = _tables_one_set
    try:
        nc = bacc.Bacc("TRN2", target_bir_lowering=False, debug=False,
                       num_devices=H)
        f32 = mybir.dt.float32
        f16 = mybir.dt.float16
        in_d = nc.dram_tensor("inp", [P, 2 * T * D + T], f32,
                              kind="ExternalInput")
        inv_d = nc.dram_tensor("inpv", [P, T * D], f16, kind="ExternalInput")
        out_d = nc.dram_tensor("out", [P, T * D], f32, kind="ExternalOutput")
        with tile.TileContext(nc) as tc:
            with ExitStack() as ctx:
                _emit(ctx, tc, in_d[:], inv_d[:], out_d[:])
        nc.compile()
    finally:
        bacc.get_activation_tables = orig_tables
    _CACHED["nc"] = nc
    return nc


def _pack(q, k, v, klen, h):
    # [512, 32] -> [128, 4*32] with col t*32+d = row t*128+p
    def rows(x):
        return np.ascontiguousarray(
            x.reshape(T, P, D).transpose(1, 0, 2).reshape(P, T * D))
    kl = np.ascontiguousarray(klen.reshape(T, P).T)  # [128, 4]
    kq = np.concatenate(
        [rows(k[0, :, h, :]), rows(q[0, :, h, :]), kl], axis=1)
    return (kq.astype(np.float32), rows(v[0, :, h, :]).astype(np.float16))


def kernel(queries, keys, values, attn_mask, query_lengths, key_lengths,
           _want_profile=False, **_ignored):
    nc = _build()
    q = np.asarray(queries, dtype=np.float32)
    k = np.asarray(keys, dtype=np.float32)
    v = np.asarray(values, dtype=np.float32)
    klen = np.asarray(key_lengths, dtype=np.float32)

    packed = [_pack(q, k, v, klen, h) for h in range(H)]
    in_maps = [{"inp": kq, "inpv": vv} for kq, vv in packed]
    res = run_bass_kernel_spmd(nc, in_maps, list(range(H)),
                               trace=_want_profile)
    # [128, 128] -> [512, 32]
    outs = [
        np.asarray(res.results[h]["out"], dtype=np.float32)
        .reshape(P, T, D).transpose(1, 0, 2).reshape(L, D)
        for h in range(H)
    ]
    out = np.stack(outs, axis=1)[None]
    if _want_profile:
        return out.astype(np.float32), res
    return out.astype(np.float32)


# revision 25
# speedup vs baseline: 1.0993x; 1.0504x over previous
"""Trainium2 Bass kernel for nn_LinearSoftmaxAttention (second-order linear attention).

Math (per batch n, head h; L == S, D == M):
    Q = LN(queries)                       [L,D]
    K = LN(keys) / (3*sqrt(D)) * klen     [S,D]
    psumA  = Kn^T [V | 1 | 1 | Kn]        [D, 2D+2] = [KV | Ksum | Ksum | KK]
    Qsum2  = Qn^T Qn                      [D,D]
    psumC  = Q @ sbA                      [L, 2D+2] = [order1 | norm1 | . | u]
    psumD  = K @ (0.5*Qsum2)              [S,D]
    ch     = rowsum(psumD * K)            [S]     (= 0.5 * c)
    nrm    = norm1 + 0.5*rowsum(u * Q)    [L]
    out    = (order1 + ch*V) / nrm[:,None]

Sharding: one (n,h) pair per NeuronCore -> 8 heads over 8 cores, no collectives.
All matmuls in f16 (inputs are converted host-side; rel-err budget is 2e-2).
Transposes of [Kn|Qn] are done per 128-row chunk on the PE ([128,64] -> [64,128]),
giving kT on partitions 0:32 and qT on partitions 32:64 so that the d-contraction
matmuls (C and D) run as row-group-tiled matmuls without any cross-partition moves.
"""

from contextlib import ExitStack

import numpy as np
import ml_dtypes

import concourse.bacc as bacc
import concourse.mybir as mybir
from concourse import tile
from concourse.bass_utils import run_bass_kernel_spmd
from concourse.masks import make_identity

# Problem constants (hardcoded per harness contract).
L = 512  # query length == key length
D = 32   # head dim == value dim
H = 8    # heads
P = 128  # SBUF partitions
T = L // P  # 4 row-chunks of 128
ALPHA = 3.0
LN_EPS = 1e-5
_INV_C2 = float(ALPHA * ALPHA * D)  # 1/c^2 = 288  (K scale folded into sqrt arg)

# dram input layout (f16 cols): k | q | klen | v
_KOFF, _QOFF, _LOFF, _VOFF = 0, T * D, 2 * T * D, 2 * T * D + T
NCOL = 3 * T * D + T  # 388

# work tile free-dim layout per chunk (f16, 4B-aligned slices):
#   [ V(0:32) | one(32) one(33) | Kn(34:66) | Qn(66:98) | pad ]
_WV, _W1, _WK, _WQ, _WW = 0, D, D + 2, 2 * D + 2, 3 * D + 4  # 0,32,34,66,100
CAT = 2 * D + 2  # 66: A-matmul rhs/psum width [KV | Ksum Ksum | KK]

_SUB = mybir.AluOpType.subtract
_MUL = mybir.AluOpType.mult
_ADD = mybir.AluOpType.add
_BYP = mybir.AluOpType.bypass
_AX = mybir.AxisListType.X


def _emit(ctx: ExitStack, tc: tile.TileContext, in_d, inv_d, out_d):
    nc = tc.nc
    f32 = mybir.dt.float32
    f16 = mybir.dt.float16
    ACT = mybir.ActivationFunctionType

    consts = ctx.enter_context(tc.tile_pool(name="consts", bufs=1))
    sbuf = ctx.enter_context(tc.tile_pool(name="sbuf", bufs=1))
    psum = ctx.enter_context(tc.tile_pool(name="psum", bufs=1, space="PSUM"))
    psum_tr = ctx.enter_context(tc.tile_pool(name="psum_tr", bufs=2, space="PSUM"))

    identity = consts.tile([P, P], f16)
    make_identity(nc, identity[:])
    dummy = consts.tile([P, 1], f32)
    eps_b = consts.tile([P, 1], f32)
    # Dependency-free Sqrt issued first so Bacc's hoisted act-table load
    # overlaps the input DMA instead of sitting on the LN critical path.
    nc.scalar.activation(dummy[:], nc.const_aps.tensor(0.0, (P, 1)), ACT.Sqrt)
    nc.gpsimd.memset(eps_b[:], LN_EPS)

    # ---- input DMAs; raw = [k | q | klen] fp32, v (f16) lands in work ----
    raw = sbuf.tile([P, 2 * T * D + T], f32)  # [128, 260]
    work = sbuf.tile([P, T, _WW], f16)
    nc.gpsimd.memset(work[:, :, _W1 : _W1 + 2], 1.0)
    nc.sync.dma_start(raw[:], in_d[:])
    nc.scalar.dma_start(
        work[:, :, _WV : _WV + D],
        inv_d.rearrange("p (t d) -> p t d", d=D),
    )
    k_raw = raw[:, 0 : T * D].rearrange("p (t d) -> p t d", d=D)
    q_raw = raw[:, T * D : 2 * T * D].rearrange("p (t d) -> p t d", d=D)
    klen = raw[:, 2 * T * D : 2 * T * D + T]  # [128, 4] f32
    kq_g = raw[:, 0 : 2 * T * D].rearrange("p (g d) -> p g d", d=D)  # 8 groups

    # ---- LN stats: s = rowsum(x), ss = rowsum(x^2) per 32-col group ----
    # (k groups 0:4, q groups 4:8). var*32 = ss - s^2/32.
    G = 2 * T
    sq = sbuf.tile([P, 2 * T * D], f16)
    nc.scalar.activation(sq[:], raw[:, 0 : 2 * T * D], ACT.Square)
    s_ = sbuf.tile([P, G], f32)
    ss = sbuf.tile([P, G], f32)
    v32 = sbuf.tile([P, G], f32)
    rs = sbuf.tile([P, G], f32)   # 1/std (k part includes klen/alpha factors)
    nmu = sbuf.tile([P, G], f32)  # -mean * rs
    std = sbuf.tile([P, G], f32)
    klenp = sbuf.tile([P, T], f32)  # klen / (alpha*sqrt(D))
    nc.vector.tensor_reduce(s_[:], kq_g, axis=_AX, op=_ADD)
    nc.gpsimd.tensor_scalar(out=klenp[:], in0=klen, scalar1=1.0 / _INV_C2**0.5,
                            scalar2=None, op0=_MUL)
    # v32 scratch = -(s/32)*s
    nc.vector.scalar_tensor_tensor(out=v32[:], in0=s_[:], scalar=-1.0 / D,
                                   in1=s_[:], op0=_MUL, op1=_MUL)
    nc.vector.tensor_reduce(ss[:], sq[:].rearrange("p (g d) -> p g d", d=D),
                            axis=_AX, op=_ADD)
    nc.vector.tensor_add(v32[:], v32[:], ss[:])
    # std = sqrt(var + eps) for all 8 groups in one ACT call
    nc.scalar.activation(std[:], v32[:], ACT.Sqrt, scale=1.0 / D, bias=eps_b[:])
    nc.vector.reciprocal(rs[:], std[:])
    nc.vector.tensor_mul(rs[:, 0:T], rs[:, 0:T], klenp[:])
    nc.vector.scalar_tensor_tensor(out=nmu[:], in0=s_[:], scalar=-1.0 / D,
                                   in1=rs[:], op0=_MUL, op1=_MUL)

    # ---- LN applies x*rs + nmu, chunk-pair halves for earlier pipelining:
    # k01 on DVE (broadcast TT pair), k23 on ACT (per-chunk Identity),
    # q01 on GpSimd (broadcast), q23 on DVE ----
    tmpk = sbuf.tile([P, 2, D], f32)
    tmpq = sbuf.tile([P, 2, D], f32)

    def bcast(src, lo):
        return src[:, lo : lo + 2, None].broadcast_to((P, 2, D))

    nc.vector.tensor_mul(tmpk[:], k_raw[:, 0:2, :], bcast(rs, 0))
    nc.vector.tensor_add(work[:, 0:2, _WK : _WK + D], tmpk[:], bcast(nmu, 0))
    for t in (2, 3):
        nc.scalar.activation(work[:, t, _WK : _WK + D], k_raw[:, t, :],
                             ACT.Identity, scale=rs[:, t : t + 1],
                             bias=nmu[:, t : t + 1])
    nc.gpsimd.tensor_mul(tmpq[:], q_raw[:, 0:2, :], bcast(rs, T))
    nc.gpsimd.tensor_add(work[:, 0:2, _WQ : _WQ + D], tmpq[:], bcast(nmu, T))
    tmpq2 = sbuf.tile([P, 2, D], f32)
    nc.vector.tensor_mul(tmpq2[:], q_raw[:, 2:4, :], bcast(rs, T + 2))
    nc.vector.tensor_add(work[:, 2:4, _WQ : _WQ + D], tmpq2[:],
                         bcast(nmu, T + 2))

    # ---- PE: A (into psum partitions 32:64), B, [Kn|Qn] transposes; the
    # transposes land chunk-pairs in one psum bank so each pair needs one
    # evacuation copy ----
    psumA = psum.tile([2 * D, CAT], f32)      # rows 32:64 used
    psumB = psum.tile([D, D], f32)
    kqT = sbuf.tile([2 * D, T, P], f16)      # rows 0:32 kT, 32:64 qT
    sbA = sbuf.tile([2 * D, CAT], f16)
    sbB = sbuf.tile([D, D], f16)
    pT = [None, None]
    for t in range(T):
        st, sp = (t == 0), (t == T - 1)
        nc.tensor.matmul(psumA[D : 2 * D, :], work[:, t, _WK : _WK + D],
                         work[:, t, 0:CAT], start=st, stop=sp,
                         tile_position=(0, D))
        nc.tensor.matmul(psumB[:], work[:, t, _WQ : _WQ + D],
                         work[:, t, _WQ : _WQ + D], start=st, stop=sp)
        if t % 2 == 0:
            ptile = psum_tr.tile([2 * D, 2, P], f16, tag="ptr")
            pT[t // 2] = ptile
        nc.tensor.transpose(pT[t // 2][:, t % 2, :],
                            work[:, t, _WK : _WK + 2 * D], identity[:])
        if t == 1:
            nc.scalar.copy(kqT[:, 0:2, :], pT[0][:])
        elif t == 3:
            nc.vector.tensor_copy(kqT[:, 2:4, :], pT[1][:])
    nc.scalar.activation(sbB[:], psumB[:], ACT.Copy, scale=0.5)
    nc.vector.tensor_copy(sbA[D : 2 * D, :], psumA[D : 2 * D, :])

    # ---- PE: D and C (row-group-tiled over d), D first per pair so the
    # ch epilogue can start as soon as psumD is complete ----
    psumD = psum.tile([P, T, D], f32)
    psumC = psum.tile([P, T, CAT], f32)
    for t in (0, 1):
        nc.tensor.matmul(psumD[:, t, :], kqT[0:D, t, :], sbB[:],
                         start=True, stop=True)
    for t in (0, 1):
        nc.tensor.matmul(psumC[:, t, :], kqT[D : 2 * D, t, :], sbA[D : 2 * D, :],
                         start=True, stop=True, tile_position=(D, 0))
    for t in (2, 3):
        nc.tensor.matmul(psumD[:, t, :], kqT[0:D, t, :], sbB[:],
                         start=True, stop=True)
    for t in (2, 3):
        nc.tensor.matmul(psumC[:, t, :], kqT[D : 2 * D, t, :], sbA[D : 2 * D, :],
                         start=True, stop=True, tile_position=(D, 0))

    # ---- epilogue ----
    # ch = rowsum(psumD * Kn)  (starts while C matmuls still run)
    e1 = sbuf.tile([P, T, D], f32)
    ch = sbuf.tile([P, T], f32)
    nc.vector.tensor_mul(e1[:], psumD[:], work[:, :, _WK : _WK + D])
    nc.vector.tensor_reduce(ch[:], e1[:], axis=_AX, op=_ADD)
    # nrm = norm1 + 0.5*rowsum(u * Qn); rnorm = 1/nrm
    e2 = sbuf.tile([P, T, D], f32)
    nrm = sbuf.tile([P, T], f32)
    nc.vector.tensor_mul(e2[:], psumC[:, :, _WK:CAT], work[:, :, _WQ : _WQ + D])
    nc.vector.tensor_reduce(nrm[:], e2[:], axis=_AX, op=_ADD)
    nc.vector.scalar_tensor_tensor(out=nrm[:], in0=nrm[:], scalar=0.5,
                                   in1=psumC[:, :, D], op0=_MUL, op1=_ADD)
    nc.vector.reciprocal(nrm[:], nrm[:])
    # out = (V*ch + order1) * rnorm, all chunks at once via broadcast TTs:
    # f1 = V*ch_b (GpSimd, as soon as ch lands), f2 = f1 + order1 (DVE),
    # out = f2 * rnorm_b (DVE), single output DMA.
    f1 = sbuf.tile([P, T, D], f32)
    f2 = sbuf.tile([P, T, D], f32)
    out_sb = sbuf.tile([P, T, D], f32)
    ch_b = ch[:, :, None].broadcast_to((P, T, D))
    nrm_b = nrm[:, :, None].broadcast_to((P, T, D))
    nc.gpsimd.tensor_mul(f1[:], work[:, :, _WV : _WV + D], ch_b)
    nc.vector.tensor_add(f2[:], f1[:], psumC[:, :, 0:D])
    nc.vector.tensor_mul(out_sb[:], f2[:], nrm_b)
    nc.sync.dma_start(out_d[:], out_sb[:].rearrange("p t d -> p (t d)"))


_CACHED = {}


def _build():
    if "nc" in _CACHED:
        return _CACHED["nc"]
    # The walrus NEFF epilogue zeroes every semaphore up to max-sem-num, one
    # EVENT_SEMAPHORE instruction each, split across the five engines — ~6us
    # of fixed teardown at the default 256. The kernel (incl. bass-managed
    # sems at 150+) stays below 166, so cap the range.
    import concourse.bass_utils as bass_utils_mod
    if not getattr(bass_utils_mod, "_max_sem_patched", False):
        _orig_walrus_args = bass_utils_mod.get_walrus_args

        def _walrus_args_capped(*args, **kwargs):
            return _orig_walrus_args(*args, **kwargs) + ["--max-sem-num=166"]

        bass_utils_mod.get_walrus_args = _walrus_args_capped
        bass_utils_mod._max_sem_patched = True

    # Route every ACT func we use (Sqrt/Copy/Identity/Square) to the single
    # act-func-set that contains them all, so Bacc inserts ONE table load
    # instead of one per first-match set.
    import concourse.hw_specs as hw_specs
    orig_tables = hw_specs.get_activation_tables

    def _tables_one_set(module_arch):
        tabs = orig_tables(module_arch)
        keep = None
        for name, funcs in tabs.items():
            names = {str(f) for f in funcs}
            if any("Sqrt" in s and "Rsqrt" not in s for s in names):
                keep = name
                break
        if keep is None:
            return tabs
        shared = {
            mybir.ActivationFunctionType.Copy,
            mybir.ActivationFunctionType.Identity,
            mybir.ActivationFunctionType.Square,
        }
        return {
            name: (funcs if name == keep else funcs - shared)
            for name, funcs in tabs.items()
        }

    bacc.get_activation_tables = _tables_one_set
    try:
        nc = bacc.Bacc("TRN2", target_bir_lowering=False, debug=False,
                       num_devices=H)
        f32 = mybir.dt.float32
        f16 = mybir.dt.float16
        in_d = nc.dram_tensor("inp", [P, 2 * T * D + T], f32,
                              kind="ExternalInput")
        inv_d = nc.dram_tensor("inpv", [P, T * D], f16, kind="ExternalInput")
        out_d = nc.dram_tensor("out", [P, T * D], f32, kind="ExternalOutput")
        with tile.TileContext(nc) as tc:
            with ExitStack() as ctx:
                _emit(ctx, tc, in_d[:], inv_d[:], out_d[:])
        nc.compile()
    finally:
        bacc.get_activation_tables = orig_tables
    _CACHED["nc"] = nc
    return nc


def _pack(q, k, v, klen, h):
    # [512, 32] -> [128, 4*32] with col t*32+d = row t*128+p
    def rows(x):
        return np.ascontiguousarray(
            x.reshape(T, P, D).transpose(1, 0, 2).reshape(P, T * D))
    kl = np.ascontiguousarray(klen.reshape(T, P).T)  # [128, 4]
    kq = np.concatenate(
        [rows(k[0, :, h, :]), rows(q[0, :, h, :]), kl], axis=1)
    return (kq.astype(np.float32), rows(v[0, :, h, :]).astype(np.float16))


def kernel(queries, keys, values, attn_mask, query_lengths, key_lengths,
           _want_profile=False, **_ignored):
    nc = _build()
    q = np.asarray(queries, dtype=np.float32)
    k = np.asarray(keys, dtype=np.float32)
    v = np.asarray(values, dtype=np.float32)
    klen = np.asarray(key_lengths, dtype=np.float32)

    packed = [_pack(q, k, v, klen, h) for h in range(H)]
    in_maps = [{"inp": kq, "inpv": vv} for kq, vv in packed]
    res = run_bass_kernel_spmd(nc, in_maps, list(range(H)),
                               trace=_want_profile)
    # [128, 128] -> [512, 32]
    outs = [
        np.asarray(res.results[h]["out"], dtype=np.float32)
        .reshape(P, T, D).transpose(1, 0, 2).reshape(L, D)
        for h in range(H)
    ]
    out = np.stack(outs, axis=1)[None]
    if _want_profile:
        return out.astype(np.float32), res
    return out.astype(np.float32)
